# revision 1
# baseline (speedup 1.0000x reference)
"""Convex_f forward on 8 trn2 NeuronCores (pure data parallel over batch).

Math: with y = x + param and the interior 3-point stencils
  Dy[i]    = -y[i-1] + 2 y[i] - y[i+1]          (0 at i = 0, N-1)
  mid_y[i] = 0.5 (y[i-1] + y[i+1])
the reference computes out = y - (Dy > 0) * (y - mid_y) - param.
Since y - mid_y = 0.5 * Dy on the interior, this collapses to
  out[i] = x[i] - relu(ctr - 0.5*up - 0.5*dn)   for 0 < i < N-1
  out[i] = x[i]                                  at i = 0, N-1.

The boundary case is folded into the interior formula by padding each
batch with a halo row at both N-ends host-side: x_halo = +1e30 and
param_halo = 0, so y_halo = +1e30 and relu(ctr - 0.5*y_halo - ...) = 0.

Per-core layout: partition p holds J=64 consecutive n-rows (x16 K) per
batch, so the stencil shift is a free-dim offset of K elements and every
DMA is one large transfer with 4KiB+ contiguous runs per partition.

Strategies:
  accum    — load x (HWDGE), copy x->y on ScalarE, land param directly
             into y via SWDGE accum-add DMA. No param tile, no gpsimd
             compute; DVE does the two stencil STTs + final subtract.
  gpsimd_y — load x and param plainly; y = x + param on GpSimd.
"""

import os

import numpy as np

B, N, K = 256, 8192, 16
NCORES = 8
BPC = B // NCORES  # 32 batches per core
P = 128
J = N // P         # 64 n-rows per partition per batch
NP = N + 2         # padded rows per batch
FHB = (J + 2) * K  # 1056 haloed free elems per batch per partition
FIB = J * K        # 1024 interior free elems per batch per partition
BIG = 1.0e30

STRATEGY = os.environ.get("CONVEX_STRATEGY", "pe_y")
BPI = int(os.environ.get("CONVEX_BPI", "1"))     # batches per iteration
BUFS = int(os.environ.get("CONVEX_BUFS", "6"))
PIPE = int(os.environ.get("CONVEX_PIPE", "1"))   # sw-pipeline the tail op
WARM = int(os.environ.get("CONVEX_WARM", "0"))   # first iters y-add on DVE

_cache = {}

# Results of the last hardware run (BassKernelResults); test harnesses can
# read exec_time_ns etc. from here after calling kernel().
LAST_RESULTS = None


def _build_nc():
    import concourse.bacc as bacc
    import concourse.bass as bass
    import concourse.mybir as mybir
    from concourse.tile import TileContext

    f32 = mybir.dt.float32
    AO = mybir.AluOpType
    AF = mybir.ActivationFunctionType
    FH = BPI * FHB
    FI = BPI * FIB

    nc = bacc.Bacc()
    x_d = nc.dram_tensor("x", [BPC, NP, K], f32, kind="ExternalInput")
    p_d = nc.dram_tensor("p", [BPC, NP, K], f32, kind="ExternalInput")
    o_d = nc.dram_tensor("o", [BPC, N, K], f32, kind="ExternalOutput")

    def halo_ap(handle, b0):
        # [p, q, f]: partition p reads padded rows [p*J, p*J + J + 2) of
        # batches b0..b0+BPI-1 (overlapping reads across partitions).
        return bass.AP(handle, b0 * NP * K, [[J * K, P], [NP * K, BPI], [1, FHB]])

    def out_ap(handle, b0):
        return bass.AP(handle, b0 * N * K, [[J * K, P], [N * K, BPI], [1, FIB]])

    n_iter = BPC // BPI
    if STRATEGY == "pe_y":
        return _build_pe_y(nc, bass, mybir, x_d, p_d, o_d, halo_ap, out_ap)
    with TileContext(nc) as tc:
        with tc.tile_pool(name="io", bufs=BUFS) as pool:
            # stage A state carried to the delayed tail stage
            pend = []

            def stage_a(it):
                b0 = it * BPI
                x_t = pool.tile([P, FH], f32, name="x_t")
                y_t = pool.tile([P, FH], f32, name="y_t")
                d_t = pool.tile([P, FI], f32, name="d_t")
                if STRATEGY == "dve_y2":
                    # 3 tiles only: p lands in y_t, y-add in place, output
                    # in place over x_t's interior view.
                    nc.sync.dma_start(x_t[:], halo_ap(x_d, b0))
                    nc.sync.dma_start(y_t[:], halo_ap(p_d, b0))
                    nc.vector.tensor_tensor(y_t[:], x_t[:], y_t[:], op=AO.add)
                    y3 = y_t.rearrange("p (q f) -> p q f", q=BPI)
                    d3 = d_t.rearrange("p (q f) -> p q f", q=BPI)
                    up = y3[:, :, 0:FIB]
                    ctr = y3[:, :, K:K + FIB]
                    dn = y3[:, :, 2 * K:2 * K + FIB]
                    nc.vector.scalar_tensor_tensor(d3[:], up, -0.5, ctr,
                                                   AO.mult, AO.add)
                    nc.vector.scalar_tensor_tensor(d3[:], dn, -0.5, d3[:],
                                                   AO.mult, AO.add)
                    nc.scalar.activation(d3[:], d3[:], AF.Relu)
                    return (it, x_t, d_t)

                nc.sync.dma_start(x_t[:], halo_ap(x_d, b0))
                if STRATEGY == "accum":
                    # y = x (ScalarE copy), then y += param via SWDGE accum
                    nc.scalar.copy(y_t[:], x_t[:])
                    nc.gpsimd.dma_start(
                        y_t[:], halo_ap(p_d, b0), accum_op=AO.add
                    )
                elif STRATEGY == "dve_y":
                    # y = x + param on DVE, in place over the param tile
                    # (gpsimd elementwise stalls DVE via the shared SBUF
                    # port lock, so gpsimd does no compute at all here)
                    nc.sync.dma_start(y_t[:], halo_ap(p_d, b0))
                    nc.vector.tensor_tensor(y_t[:], x_t[:], y_t[:], op=AO.add)
                else:
                    p_t = pool.tile([P, FH], f32, name="p_t")
                    nc.sync.dma_start(p_t[:], halo_ap(p_d, b0))
                    nc.gpsimd.tensor_tensor(y_t[:], x_t[:], p_t[:], op=AO.add)

                y3 = y_t.rearrange("p (q f) -> p q f", q=BPI)
                d3 = d_t.rearrange("p (q f) -> p q f", q=BPI)
                up = y3[:, :, 0:FIB]
                ctr = y3[:, :, K:K + FIB]
                dn = y3[:, :, 2 * K:2 * K + FIB]

                # e = ctr - 0.5*up ; d = e - 0.5*dn = ctr - 0.5*(up + dn)
                nc.vector.scalar_tensor_tensor(d3[:], up, -0.5, ctr, AO.mult, AO.add)
                nc.vector.scalar_tensor_tensor(d3[:], dn, -0.5, d3[:], AO.mult, AO.add)
                # r = relu(d) in place on ScalarE
                nc.scalar.activation(d3[:], d3[:], AF.Relu)
                return (it, x_t, d_t)

            def stage_b(state):
                it, x_t, d_t = state
                b0 = it * BPI
                x3 = x_t.rearrange("p (q f) -> p q f", q=BPI)
                d3 = d_t.rearrange("p (q f) -> p q f", q=BPI)
                xc = x3[:, :, K:K + FIB]
                if STRATEGY == "dve_y2":
                    # out = x - relu(d), in place over x's interior view
                    nc.vector.tensor_tensor(xc, xc, d3[:], op=AO.subtract)
                    nc.scalar.dma_start(out_ap(o_d, b0), xc)
                    return
                o_t = pool.tile([P, FI], f32, name="o_t")
                o3 = o_t.rearrange("p (q f) -> p q f", q=BPI)
                # out = x - relu(d)
                nc.vector.tensor_tensor(o3[:], xc, d3[:], op=AO.subtract)
                # stores go out on the ACT HWDGE ring so a store waiting on
                # o_t can't head-of-line-block the next loads on the SP ring
                nc.scalar.dma_start(out_ap(o_d, b0), o_t[:])

            for it in range(n_iter):
                pend.append(stage_a(it))
                if len(pend) > PIPE:
                    stage_b(pend.pop(0))
            for s in pend:
                stage_b(s)
    nc.finalize()
    return nc


def _build_pe_y(nc, bass, mybir, x_d, p_d, o_d, halo_ap, out_ap):
    """y = x + param on the TensorEngine (identity-matmul accumulate into
    PSUM), then per batch on DVE (each op reads at most one PSUM operand):
        u1 = 0.5*y_up - p_ctr
        u  = 0.5*y_dn + u1         (= x_ctr - d, with d the relu argument)
        o  = min(x_ctr, u)         (= x - relu(d))
    No relu, no PSUM->SBUF copy, no y-add on DVE. Loads split over the SP
    and ACT HWDGE rings; stores on SWDGE (GpSimd is otherwise idle).
    """
    import numpy as np
    from concourse.tile import TileContext

    f32 = mybir.dt.float32
    AO = mybir.AluOpType
    FH = BPI * FHB
    FI = BPI * FIB
    n_iter = BPC // BPI

    # bf16 identity is exact (1.0/0.0) and enables fast-weight-load;
    # f32 identity is the proven default
    ident_d = ident_bf_d = None
    if os.environ.get("CONVEX_IDENT_BF16"):
        import ml_dtypes
        ident_bf_d = nc.inline_tensor(
            np.eye(P, dtype=ml_dtypes.bfloat16), name="ident"
        )
    else:
        ident_d = nc.inline_tensor(np.eye(P, dtype=np.float32), name="ident")

    with TileContext(nc) as tc:
        with (
            tc.tile_pool(name="const", bufs=1) as cpool,
            tc.tile_pool(name="io", bufs=BUFS) as pool,
            tc.tile_pool(name="ps", bufs=2, space="PSUM") as pspool,
        ):
            if ident_d is not None:
                ident_t = cpool.tile([P, P], f32, name="ident_t")
                nc.sync.dma_start(ident_t[:], ident_d.ap())
            else:
                ident_t = cpool.tile([P, P], mybir.dt.bfloat16, name="ident_t")
                nc.sync.dma_start(ident_t[:], ident_bf_d.ap())

            pend = []

            def stage_a(it):
                b0 = it * BPI
                x_t = pool.tile([P, FH], f32, name="x_t")
                p_t = pool.tile([P, FH], f32, name="p_t")
                u_t = pool.tile([P, FI], f32, name="u_t")

                nc.sync.dma_start(x_t[:], halo_ap(x_d, b0))
                nc.scalar.dma_start(p_t[:], halo_ap(p_d, b0))

                x3 = x_t.rearrange("p (q f) -> p q f", q=BPI)
                p3 = p_t.rearrange("p (q f) -> p q f", q=BPI)
                u3 = u_t.rearrange("p (q f) -> p q f", q=BPI)

                if it < WARM:
                    # first iterations: y-add on DVE so nothing waits on a
                    # cold TensorEngine chain at startup
                    y_t = pool.tile([P, FH], f32, name="y_t")
                    nc.vector.tensor_tensor(y_t[:], x_t[:], p_t[:], op=AO.add)
                    y3 = y_t.rearrange("p (q f) -> p q f", q=BPI)
                    for q in range(BPI):
                        uq = u3[:, q, :]
                        nc.vector.scalar_tensor_tensor(
                            uq, y3[:, q, 0:FIB], 0.5, p3[:, q, K:K + FIB],
                            AO.mult, AO.subtract)
                        nc.vector.scalar_tensor_tensor(
                            uq, y3[:, q, 2 * K:2 * K + FIB], 0.5, uq,
                            AO.mult, AO.add)
                    return (it, x_t, u_t)

                for q in range(BPI):
                    ps = pspool.tile([P, FHB], f32, name="ps")
                    qo = q * FHB
                    # y = x + p, accumulated on the PE per <=512-col chunk
                    for c0 in range(0, FHB, 512):
                        c1 = min(c0 + 512, FHB)
                        nc.tensor.matmul(ps[:, c0:c1], ident_t[:],
                                         x_t[:, qo + c0:qo + c1],
                                         start=True, stop=False)
                        nc.tensor.matmul(ps[:, c0:c1], ident_t[:],
                                         p_t[:, qo + c0:qo + c1],
                                         start=False, stop=True)
                    uq = u3[:, q, :]
                    # u1 = 0.5*y_up - p_ctr ; u = 0.5*y_dn + u1
                    nc.vector.scalar_tensor_tensor(
                        uq, ps[:, 0:FIB], 0.5, p3[:, q, K:K + FIB],
                        AO.mult, AO.subtract)
                    nc.vector.scalar_tensor_tensor(
                        uq, ps[:, 2 * K:2 * K + FIB], 0.5, uq,
                        AO.mult, AO.add)
                return (it, x_t, u_t)

            def stage_b(state):
                it, x_t, u_t = state
                b0 = it * BPI
                o_t = pool.tile([P, FI], f32, name="o_t")
                x3 = x_t.rearrange("p (q f) -> p q f", q=BPI)
                o3 = o_t.rearrange("p (q f) -> p q f", q=BPI)
                u3 = u_t.rearrange("p (q f) -> p q f", q=BPI)
                # o = min(x_ctr, u) = x - relu(d)
                nc.vector.tensor_tensor(o3[:], x3[:, :, K:K + FIB], u3[:],
                                        op=AO.min)
                nc.gpsimd.dma_start(out_ap(o_d, b0), o_t[:])

            for it in range(n_iter):
                pend.append(stage_a(it))
                if len(pend) > PIPE:
                    stage_b(pend.pop(0))
            for s in pend:
                stage_b(s)
    nc.finalize()
    return nc


def _pad_inputs(x, param):
    # -> per-core padded slabs, shape [NCORES, BPC, NP, K]
    x = np.ascontiguousarray(x, dtype=np.float32).reshape(NCORES, BPC, N, K)
    param = np.ascontiguousarray(param, dtype=np.float32).reshape(NCORES, BPC, N, K)
    xp = np.empty((NCORES, BPC, NP, K), dtype=np.float32)
    pp = np.empty((NCORES, BPC, NP, K), dtype=np.float32)
    xp[:, :, 1:N + 1] = x
    xp[:, :, 0] = BIG
    xp[:, :, N + 1] = BIG
    pp[:, :, 1:N + 1] = param
    pp[:, :, 0] = 0.0
    pp[:, :, N + 1] = 0.0
    return xp, pp


def kernel(x: np.ndarray, param: np.ndarray) -> np.ndarray:
    global LAST_RESULTS
    from concourse.bass_utils import run_bass_kernel_spmd

    if "nc" not in _cache:
        _cache["nc"] = _build_nc()
    nc = _cache["nc"]

    xp, pp = _pad_inputs(x, param)
    in_maps = [{"x": xp[c], "p": pp[c]} for c in range(NCORES)]

    trace = bool(os.environ.get("BASS_TRACE"))
    res = run_bass_kernel_spmd(
        nc, in_maps, core_ids=list(range(NCORES)), trace=trace
    )
    LAST_RESULTS = res
    out = np.concatenate([res.results[c]["o"] for c in range(NCORES)], axis=0)
    return out.reshape(B, N, K)



# revision 2
# speedup vs baseline: 1.3173x; 1.3173x over previous
"""Convex_f forward on 8 trn2 NeuronCores (pure data parallel over batch).

Math: with y = x + param and the interior 3-point stencils
  Dy[i]    = -y[i-1] + 2 y[i] - y[i+1]          (0 at i = 0, N-1)
  mid_y[i] = 0.5 (y[i-1] + y[i+1])
the reference computes out = y - (Dy > 0) * (y - mid_y) - param.
Since y - mid_y = 0.5 * Dy on the interior, this collapses to
  out[i] = x[i] - relu(ctr - 0.5*up - 0.5*dn)   for 0 < i < N-1
  out[i] = x[i]                                  at i = 0, N-1,
and with u = x_ctr - d = 0.5*y_up + 0.5*y_dn - p_ctr it is
  out = min(x_ctr, u).

The boundary case is folded into the interior formula by padding each
batch with a halo row at both N-ends host-side: x_halo = +1e30 and
param_halo = 0, so y_halo = +1e30 and u = 0.5e30 > x at rows 0, N-1.

The problem is HBM-bound (the only real lever is bytes moved), and the
rel-err budget (2e-2) dwarfs bf16 rounding (~4e-3), so all device I/O
is bf16: host downcasts the padded inputs, device stores bf16 output,
host upcasts. 24 MiB/core of traffic vs 48 MiB in f32.

Per-core layout: partition p holds J=64 consecutive n-rows (x16 K) per
batch, so the stencil shift is a free-dim offset of K elements and every
DMA has 2KiB contiguous runs per partition. Per-batch haloed blocks of
1056 elems are packed back-to-back in SBUF, which makes the three
stencil ops single full-tile DVE instructions (operands step-1 and
4B-aligned -> 2x packed bf16 mode): positions whose stencil window
straddles a block boundary compute garbage that is simply never stored.

Engine split (strategy b16pe, default):
  PE     y = x + p per 512-col chunk into PSUM (bf16 identity matmul)
  ACT    downcast-copy PSUM y -> bf16 SBUF (also issues p loads)
  DVE    u1 = 0.5*y_up - p_ctr ; u = 0.5*y_dn + u1 ; o = min(x_ctr, u)
  SP     x loads; GpSimd(SWDGE) stores (DVE runs 1-port mode, no lock)
Strategy b16dve instead does y = x + p on DVE (no PE/ACT/PSUM).
"""

import os

import numpy as np

B, N, K = 256, 8192, 16
NCORES = 8
BPC = B // NCORES  # 32 batches per core
P = 128
J = N // P         # 64 n-rows per partition per batch
NP = N + 2         # padded rows per batch
FHB = (J + 2) * K  # 1056 haloed free elems per batch per partition
FIB = J * K        # 1024 interior free elems per batch per partition
BIG = 1.0e30

STRATEGY = os.environ.get("CONVEX_STRATEGY", "b16pe")
BPI = int(os.environ.get("CONVEX_BPI", "4"))     # batches per iteration
BUFS = int(os.environ.get("CONVEX_BUFS", "5"))
PIPE = int(os.environ.get("CONVEX_PIPE", "1"))   # sw-pipeline the store

_cache = {}

# Results of the last hardware run (BassKernelResults); test harnesses can
# read exec_time_ns etc. from here after calling kernel().
LAST_RESULTS = None


def _build_nc():
    import ml_dtypes
    import concourse.bacc as bacc
    import concourse.bass as bass
    import concourse.mybir as mybir
    from concourse.tile import TileContext

    bf16 = mybir.dt.bfloat16
    f32 = mybir.dt.float32
    AO = mybir.AluOpType
    L = BPI * FHB           # packed haloed elems per iteration
    LV = L - 2 * K          # valid length for the fused stencil ops
    n_iter = BPC // BPI

    nc = bacc.Bacc()
    x_d = nc.dram_tensor("x", [BPC, NP, K], bf16, kind="ExternalInput")
    p_d = nc.dram_tensor("p", [BPC, NP, K], bf16, kind="ExternalInput")
    o_d = nc.dram_tensor("o", [BPC, N, K], bf16, kind="ExternalOutput")

    def halo_ap(handle, b0):
        # [p, q, f]: partition p reads padded rows [p*J, p*J + J + 2) of
        # batches b0..b0+BPI-1 (overlapping reads across partitions).
        return bass.AP(handle, b0 * NP * K, [[J * K, P], [NP * K, BPI], [1, FHB]])

    def out_ap(handle, b0):
        return bass.AP(handle, b0 * N * K, [[J * K, P], [N * K, BPI], [1, FIB]])

    use_pe = STRATEGY == "b16pe"
    if use_pe:
        ident_np = np.eye(P, dtype=ml_dtypes.bfloat16)
        ident_d = nc.inline_tensor(ident_np, name="ident")

    with TileContext(nc) as tc:
        with (
            tc.tile_pool(name="const", bufs=1) as cpool,
            tc.tile_pool(name="io", bufs=BUFS) as pool,
            tc.tile_pool(name="ps", bufs=8, space="PSUM") as pspool,
        ):
            if use_pe:
                ident_t = cpool.tile([P, P], bf16, name="ident_t")
                nc.sync.dma_start(ident_t[:], ident_d.ap())

            pend = []

            def stage_a(it):
                b0 = it * BPI
                x_t = pool.tile([P, L], bf16, name="x_t")
                p_t = pool.tile([P, L], bf16, name="p_t")
                y_t = pool.tile([P, L], bf16, name="y_t")
                u_t = pool.tile([P, L], bf16, name="u_t")

                nc.sync.dma_start(x_t[:], halo_ap(x_d, b0))
                nc.scalar.dma_start(p_t[:], halo_ap(p_d, b0))

                if use_pe:
                    # y = x + p on the PE (bf16 identity matmul into f32
                    # PSUM), downcast to bf16 SBUF on ACT per 512-chunk
                    for c0 in range(0, L, 512):
                        c1 = min(c0 + 512, L)
                        ps = pspool.tile([P, c1 - c0], f32, name="ps")
                        nc.tensor.matmul(ps[:], ident_t[:], x_t[:, c0:c1],
                                         start=True, stop=False)
                        nc.tensor.matmul(ps[:], ident_t[:], p_t[:, c0:c1],
                                         start=False, stop=True)
                        nc.scalar.copy(y_t[:, c0:c1], ps[:])
                else:
                    nc.vector.tensor_tensor(y_t[:], x_t[:], p_t[:], op=AO.add)

                # fused full-tile stencil (all operands bf16, step-1,
                # 4B-aligned -> DVE 2x packed mode); block-straddling
                # positions are garbage and never stored
                uv = u_t[:, 0:LV]
                # u1 = 0.5*y_up - p_ctr
                nc.vector.scalar_tensor_tensor(
                    uv, y_t[:, 0:LV], 0.5, p_t[:, K:K + LV],
                    AO.mult, AO.subtract)
                # u = 0.5*y_dn + u1  ( = x_ctr - d )
                nc.vector.scalar_tensor_tensor(
                    uv, y_t[:, 2 * K:2 * K + LV], 0.5, uv,
                    AO.mult, AO.add)
                # o = min(x_ctr, u) = x - relu(d), in place over u
                nc.vector.tensor_tensor(uv, x_t[:, K:K + LV], uv, op=AO.min)
                return (it, u_t)

            def stage_b(state):
                it, u_t = state
                b0 = it * BPI
                u3 = u_t.rearrange("p (q f) -> p q f", q=BPI)
                nc.gpsimd.dma_start(out_ap(o_d, b0), u3[:, :, 0:FIB])

            for it in range(n_iter):
                pend.append(stage_a(it))
                if len(pend) > PIPE:
                    stage_b(pend.pop(0))
            for s in pend:
                stage_b(s)
    nc.finalize()
    return nc


def _pad_inputs(x, param):
    # -> per-core padded bf16 slabs, shape [NCORES, BPC, NP, K]
    import ml_dtypes

    bf = ml_dtypes.bfloat16
    x = np.ascontiguousarray(x, dtype=np.float32).reshape(NCORES, BPC, N, K)
    param = np.ascontiguousarray(param, dtype=np.float32).reshape(NCORES, BPC, N, K)
    xp = np.empty((NCORES, BPC, NP, K), dtype=bf)
    pp = np.empty((NCORES, BPC, NP, K), dtype=bf)
    xp[:, :, 1:N + 1] = x.astype(bf)
    xp[:, :, 0] = bf(BIG)
    xp[:, :, N + 1] = bf(BIG)
    pp[:, :, 1:N + 1] = param.astype(bf)
    pp[:, :, 0] = 0.0
    pp[:, :, N + 1] = 0.0
    return xp, pp


def kernel(x: np.ndarray, param: np.ndarray) -> np.ndarray:
    global LAST_RESULTS
    from concourse.bass_utils import run_bass_kernel_spmd

    if "nc" not in _cache:
        _cache["nc"] = _build_nc()
    nc = _cache["nc"]

    xp, pp = _pad_inputs(x, param)
    in_maps = [{"x": xp[c], "p": pp[c]} for c in range(NCORES)]

    trace = bool(os.environ.get("BASS_TRACE"))
    res = run_bass_kernel_spmd(
        nc, in_maps, core_ids=list(range(NCORES)), trace=trace
    )
    LAST_RESULTS = res
    out = np.concatenate([res.results[c]["o"] for c in range(NCORES)], axis=0)
    return out.reshape(B, N, K).astype(np.float32)


# revision 6
# speedup vs baseline: 1.6509x; 1.2533x over previous
"""Convex_f forward on 8 trn2 NeuronCores (pure data parallel over batch).

Math: with y = x + param and the interior 3-point stencils
  Dy[i]    = -y[i-1] + 2 y[i] - y[i+1]          (0 at i = 0, N-1)
  mid_y[i] = 0.5 (y[i-1] + y[i+1])
the reference computes out = y - (Dy > 0) * (y - mid_y) - param.
Since y - mid_y = 0.5 * Dy on the interior, this collapses to
  out[i] = x[i] - relu(ctr - 0.5*up - 0.5*dn)   for 0 < i < N-1
  out[i] = x[i]                                  at i = 0, N-1,
and with u = x_ctr - d = 0.5*y_up + 0.5*y_dn - p_ctr it is
  out = min(x_ctr, u).

The boundary case is folded into the interior formula by padding each
batch with a halo row at both N-ends host-side: x_halo = +1e30 and
param_halo = 0, so y_halo = +1e30 and u = 0.5e30 > x at rows 0, N-1.

The problem is HBM-bound (the only real lever is bytes moved), and the
rel-err budget (2e-2) dwarfs bf16 rounding (~4e-3), so all device I/O
is bf16: host downcasts the padded inputs, device stores bf16 output,
host upcasts. 24 MiB/core of traffic vs 48 MiB in f32.

Per-core layout: partition p holds J=64 consecutive n-rows (x16 K) per
batch, so the stencil shift is a free-dim offset of K elements and every
DMA has 2KiB contiguous runs per partition. Per-batch haloed blocks of
1056 elems are packed back-to-back in SBUF, which makes the three
stencil ops single full-tile DVE instructions (operands step-1 and
4B-aligned -> 2x packed bf16 mode): positions whose stencil window
straddles a block boundary compute garbage that is simply never stored.

Engine split (strategy b16pe, default):
  PE     h = 0.5*(x + p) per 512-col chunk into PSUM (0.5*I matmul; the
         0.5 lives in the weights because DVE scalar_tensor_tensor runs
         at 1x -- only plain tensor_tensor ops get the 2x packed mode)
  ACT    downcast-copy PSUM h -> bf16 SBUF (also issues p loads)
  DVE    t = h_up + h_dn ; u = t - p_ctr ; o = min(x_ctr, u)
  SP     x loads; GpSimd(SWDGE) stores (DVE runs 1-port mode, no lock)
Strategy b16dve instead does y = x + p on DVE (no PE/ACT/PSUM).
"""

import os

import numpy as np

B, N, K = 256, 8192, 16
NCORES = 8
BPC = B // NCORES  # 32 batches per core
P = 128
J = N // P         # 64 n-rows per partition per batch
NP = N + 2         # padded rows per batch
FHB = (J + 2) * K  # 1056 haloed free elems per batch per partition
FIB = J * K        # 1024 interior free elems per batch per partition
BIG = 1.0e30

STRATEGY = os.environ.get("CONVEX_STRATEGY", "b16pe")
BPI = int(os.environ.get("CONVEX_BPI", "4"))     # batches per iteration
BUFS = int(os.environ.get("CONVEX_BUFS", "5"))
PIPE = int(os.environ.get("CONVEX_PIPE", "1"))   # sw-pipeline the store

_cache = {}

# Results of the last hardware run (BassKernelResults); test harnesses can
# read exec_time_ns etc. from here after calling kernel().
LAST_RESULTS = None


def _build_nc():
    import ml_dtypes
    import concourse.bacc as bacc
    import concourse.bass as bass
    import concourse.mybir as mybir
    from concourse.tile import TileContext

    bf16 = mybir.dt.bfloat16
    f32 = mybir.dt.float32
    AO = mybir.AluOpType
    L = BPI * FHB           # packed haloed elems per iteration
    LV = L - 2 * K          # valid length for the fused stencil ops
    n_iter = BPC // BPI

    nc = bacc.Bacc()
    x_d = nc.dram_tensor("x", [BPC, NP, K], bf16, kind="ExternalInput")
    p_d = nc.dram_tensor("p", [BPC, NP, K], bf16, kind="ExternalInput")
    o_d = nc.dram_tensor("o", [BPC, N, K], bf16, kind="ExternalOutput")

    def halo_ap(handle, b0):
        # [p, q, f]: partition p reads padded rows [p*J, p*J + J + 2) of
        # batches b0..b0+BPI-1 (overlapping reads across partitions).
        return bass.AP(handle, b0 * NP * K, [[J * K, P], [NP * K, BPI], [1, FHB]])

    def out_ap(handle, b0):
        return bass.AP(handle, b0 * N * K, [[J * K, P], [N * K, BPI], [1, FIB]])

    use_pe = STRATEGY == "b16pe"
    if use_pe:
        ident_np = (0.5 * np.eye(P)).astype(ml_dtypes.bfloat16)
        ident_d = nc.inline_tensor(ident_np, name="ident")

    with TileContext(nc) as tc:
        with (
            tc.tile_pool(name="const", bufs=1) as cpool,
            tc.tile_pool(name="io", bufs=BUFS) as pool,
            tc.tile_pool(name="ps", bufs=8, space="PSUM") as pspool,
        ):
            if use_pe:
                ident_t = cpool.tile([P, P], bf16, name="ident_t")
                nc.sync.dma_start(ident_t[:], ident_d.ap())

            pend = []

            def stage_a(it):
                b0 = it * BPI
                x_t = pool.tile([P, L], bf16, name="x_t")
                p_t = pool.tile([P, L], bf16, name="p_t")
                y_t = pool.tile([P, L], bf16, name="y_t")
                u_t = pool.tile([P, L], bf16, name="u_t")

                nc.sync.dma_start(x_t[:], halo_ap(x_d, b0))
                nc.scalar.dma_start(p_t[:], halo_ap(p_d, b0))

                if use_pe:
                    # h = 0.5*(x + p) on the PE (0.5*I matmul into f32
                    # PSUM), downcast to bf16 SBUF on ACT per 512-chunk
                    for c0 in range(0, L, 512):
                        c1 = min(c0 + 512, L)
                        ps = pspool.tile([P, c1 - c0], f32, name="ps")
                        nc.tensor.matmul(ps[:], ident_t[:], x_t[:, c0:c1],
                                         start=True, stop=False)
                        nc.tensor.matmul(ps[:], ident_t[:], p_t[:, c0:c1],
                                         start=False, stop=True)
                        nc.scalar.copy(y_t[:, c0:c1], ps[:])
                    # fused full-tile stencil (all operands bf16, step-1,
                    # 4B-aligned, plain TT -> DVE 2x packed mode);
                    # block-straddling garbage is never stored
                    uv = u_t[:, 0:LV]
                    # t = h_up + h_dn = 0.5*(y_up + y_dn)
                    nc.vector.tensor_tensor(uv, y_t[:, 0:LV],
                                            y_t[:, 2 * K:2 * K + LV],
                                            op=AO.add)
                    # u = t - p_ctr  ( = x_ctr - d )
                    nc.vector.tensor_tensor(uv, uv, p_t[:, K:K + LV],
                                            op=AO.subtract)
                    # o = min(x_ctr, u) = x - relu(d), in place over u
                    nc.vector.tensor_tensor(uv, x_t[:, K:K + LV], uv,
                                            op=AO.min)
                    return (it, u_t)

                # all-DVE fallback: y = x + p, then STT stencil (STTs run
                # at 1x -- kept only as a correctness reference)
                nc.vector.tensor_tensor(y_t[:], x_t[:], p_t[:], op=AO.add)
                uv = u_t[:, 0:LV]
                nc.vector.scalar_tensor_tensor(
                    uv, y_t[:, 0:LV], 0.5, p_t[:, K:K + LV],
                    AO.mult, AO.subtract)
                nc.vector.scalar_tensor_tensor(
                    uv, y_t[:, 2 * K:2 * K + LV], 0.5, uv, AO.mult, AO.add)
                nc.vector.tensor_tensor(uv, x_t[:, K:K + LV], uv, op=AO.min)
                return (it, u_t)

            def stage_b(state):
                it, u_t = state
                b0 = it * BPI
                u3 = u_t.rearrange("p (q f) -> p q f", q=BPI)
                nc.gpsimd.dma_start(out_ap(o_d, b0), u3[:, :, 0:FIB])

            for it in range(n_iter):
                pend.append(stage_a(it))
                if len(pend) > PIPE:
                    stage_b(pend.pop(0))
            for s in pend:
                stage_b(s)
    nc.finalize()
    return nc


def _pad_inputs(x, param):
    # -> per-core padded bf16 slabs, shape [NCORES, BPC, NP, K]
    import ml_dtypes

    bf = ml_dtypes.bfloat16
    x = np.ascontiguousarray(x, dtype=np.float32).reshape(NCORES, BPC, N, K)
    param = np.ascontiguousarray(param, dtype=np.float32).reshape(NCORES, BPC, N, K)
    xp = np.empty((NCORES, BPC, NP, K), dtype=bf)
    pp = np.empty((NCORES, BPC, NP, K), dtype=bf)
    xp[:, :, 1:N + 1] = x.astype(bf)
    xp[:, :, 0] = bf(BIG)
    xp[:, :, N + 1] = bf(BIG)
    pp[:, :, 1:N + 1] = param.astype(bf)
    pp[:, :, 0] = 0.0
    pp[:, :, N + 1] = 0.0
    return xp, pp


def kernel(x: np.ndarray, param: np.ndarray) -> np.ndarray:
    global LAST_RESULTS
    from concourse.bass_utils import run_bass_kernel_spmd

    if "nc" not in _cache:
        _cache["nc"] = _build_nc()
    nc = _cache["nc"]

    xp, pp = _pad_inputs(x, param)
    in_maps = [{"x": xp[c], "p": pp[c]} for c in range(NCORES)]

    trace = bool(os.environ.get("BASS_TRACE"))
    res = run_bass_kernel_spmd(
        nc, in_maps, core_ids=list(range(NCORES)), trace=trace
    )
    LAST_RESULTS = res
    out = np.concatenate([res.results[c]["o"] for c in range(NCORES)], axis=0)
    return out.reshape(B, N, K).astype(np.float32)


# revision 11
# speedup vs baseline: 1.8846x; 1.1416x over previous
"""Convex_f forward on 8 trn2 NeuronCores (pure data parallel over batch).

Math: with y = x + param and the interior 3-point stencils
  Dy[i]    = -y[i-1] + 2 y[i] - y[i+1]          (0 at i = 0, N-1)
  mid_y[i] = 0.5 (y[i-1] + y[i+1])
the reference computes out = y - (Dy > 0) * (y - mid_y) - param.
Since y - mid_y = 0.5 * Dy on the interior, this collapses to
  out[i] = x[i] - relu(ctr - 0.5*up - 0.5*dn)   for 0 < i < N-1
  out[i] = x[i]                                  at i = 0, N-1,
and with u = x_ctr - d = 0.5*y_up + 0.5*y_dn - p_ctr it is
  out = min(x_ctr, u).

The boundary case is folded into the interior formula by padding each
batch with a halo row at both N-ends host-side: x_halo = +1e30 and
param_halo = 0, so y_halo = +1e30 and u = 0.5e30 > x at rows 0, N-1.

The problem is HBM-bound (the only real lever is bytes moved), and the
rel-err budget (2e-2) dwarfs bf16 rounding (~4e-3), so all device I/O
is bf16: host downcasts the padded inputs, device stores bf16 output,
host upcasts. 24 MiB/core of traffic vs 48 MiB in f32.

Per-core layout: partition p holds J=64 consecutive n-rows (x16 K) per
batch, so the stencil shift is a free-dim offset of K elements and every
DMA has 2KiB contiguous runs per partition. Per-batch haloed blocks of
1056 elems are packed back-to-back in SBUF, which makes the three
stencil ops single full-tile DVE instructions (operands step-1 and
4B-aligned -> 2x packed bf16 mode): positions whose stencil window
straddles a block boundary compute garbage that is simply never stored.

Engine split (strategy b16pe, default):
  PE     h = 0.5*(x + p) per 512-col chunk into PSUM (0.5*I matmul; the
         0.5 lives in the weights because DVE scalar_tensor_tensor runs
         at 1x -- only plain tensor_tensor ops get the 2x packed mode)
  ACT    downcast-copy PSUM h -> bf16 SBUF
  DVE    t = h_up + h_dn ; u = t - p_ctr ; o = min(x_ctr, u)
  SP     x AND p loads (a load issued on the ACT ring would queue
         behind the semaphore-waiting copies -- in-order sequencer --
         and serialize the load pipeline behind compute)
  GpSimd SWDGE stores (DVE runs 1-port mode, no descriptor-ring lock)
The DVE stencil + store are sub-chunked (CHUNKS per iteration) so the
first store issues before the whole iteration's stencil is done and the
pipeline tail stays short.
Strategy b16dve instead does y = x + p on DVE (no PE/ACT/PSUM).
"""

import os

import numpy as np

B, N, K = 256, 8192, 16
NCORES = 8
BPC = B // NCORES  # 32 batches per core
P = 128
J = N // P         # 64 n-rows per partition per batch
NP = N + 2         # padded rows per batch
FHB = (J + 2) * K  # 1056 haloed free elems per batch per partition
FIB = J * K        # 1024 interior free elems per batch per partition
BIG = 1.0e30

STRATEGY = os.environ.get("CONVEX_STRATEGY", "b16pe")
BPI = int(os.environ.get("CONVEX_BPI", "4"))     # batches per iteration
BUFS = int(os.environ.get("CONVEX_BUFS", "5"))
CHUNKS = int(os.environ.get("CONVEX_CHUNKS", "2"))  # stencil+store splits

_cache = {}

# Results of the last hardware run (BassKernelResults); test harnesses can
# read exec_time_ns etc. from here after calling kernel().
LAST_RESULTS = None


def _build_nc():
    import ml_dtypes
    import concourse.bacc as bacc
    import concourse.bass as bass
    import concourse.mybir as mybir
    from concourse.tile import TileContext

    bf16 = mybir.dt.bfloat16
    f32 = mybir.dt.float32
    AO = mybir.AluOpType
    L = BPI * FHB           # packed haloed elems per iteration
    BPS = BPI // CHUNKS     # batches per stencil/store sub-chunk
    LS = BPS * FHB          # packed elems per sub-chunk
    LSV = LS - 2 * K        # valid length per sub-chunk
    n_iter = BPC // BPI

    nc = bacc.Bacc()
    x_d = nc.dram_tensor("x", [BPC, NP, K], bf16, kind="ExternalInput")
    p_d = nc.dram_tensor("p", [BPC, NP, K], bf16, kind="ExternalInput")
    o_d = nc.dram_tensor("o", [BPC, N, K], bf16, kind="ExternalOutput")

    def halo_ap(handle, b0):
        # [p, q, f]: partition p reads padded rows [p*J, p*J + J + 2) of
        # batches b0..b0+BPI-1 (overlapping reads across partitions).
        return bass.AP(handle, b0 * NP * K, [[J * K, P], [NP * K, BPI], [1, FHB]])

    def out_ap(handle, b0, nb):
        return bass.AP(handle, b0 * N * K, [[J * K, P], [N * K, nb], [1, FIB]])

    use_pe = STRATEGY == "b16pe"
    if use_pe:
        ident_np = (0.5 * np.eye(P)).astype(ml_dtypes.bfloat16)
        ident_d = nc.inline_tensor(ident_np, name="ident")

    with TileContext(nc) as tc:
        with (
            tc.tile_pool(name="const", bufs=1) as cpool,
            tc.tile_pool(name="io", bufs=BUFS) as pool,
            tc.tile_pool(name="ps", bufs=8, space="PSUM") as pspool,
        ):
            if use_pe:
                ident_t = cpool.tile([P, P], bf16, name="ident_t")
                nc.sync.dma_start(ident_t[:], ident_d.ap())

            for it in range(n_iter):
                b0 = it * BPI
                x_t = pool.tile([P, L], bf16, name="x_t")
                p_t = pool.tile([P, L], bf16, name="p_t")
                y_t = pool.tile([P, L], bf16, name="y_t")
                u_t = pool.tile([P, L], bf16, name="u_t")

                nc.sync.dma_start(x_t[:], halo_ap(x_d, b0))
                nc.sync.dma_start(p_t[:], halo_ap(p_d, b0))

                if use_pe:
                    # h = 0.5*(x + p) on the PE (0.5*I matmul into f32
                    # PSUM), downcast to bf16 SBUF on ACT per 512-chunk
                    for c0 in range(0, L, 512):
                        c1 = min(c0 + 512, L)
                        ps = pspool.tile([P, c1 - c0], f32, name="ps")
                        nc.tensor.matmul(ps[:], ident_t[:], x_t[:, c0:c1],
                                         start=True, stop=False)
                        nc.tensor.matmul(ps[:], ident_t[:], p_t[:, c0:c1],
                                         start=False, stop=True)
                        nc.scalar.copy(y_t[:, c0:c1], ps[:])
                else:
                    # all-DVE fallback: h = 0.5*(x + p) via 2x add +
                    # 4x-capable scalar multiply (STT stencil runs 1x)
                    nc.vector.tensor_tensor(y_t[:], x_t[:], p_t[:],
                                            op=AO.add)
                    nc.vector.tensor_scalar(y_t[:], y_t[:], 0.5,
                                            op=AO.mult)

                u3 = u_t.rearrange("p (q f) -> p q f", q=BPI)
                for s in range(CHUNKS):
                    o0 = s * LS
                    # fused stencil per sub-chunk (operands bf16, step-1,
                    # 4B-aligned, plain TT -> DVE 2x packed mode);
                    # block-straddling garbage is never stored
                    uv = u_t[:, o0:o0 + LSV]
                    # t = h_up + h_dn = 0.5*(y_up + y_dn)
                    nc.vector.tensor_tensor(
                        uv, y_t[:, o0:o0 + LSV],
                        y_t[:, o0 + 2 * K:o0 + 2 * K + LSV], op=AO.add)
                    # u = t - p_ctr  ( = x_ctr - d )
                    nc.vector.tensor_tensor(
                        uv, uv, p_t[:, o0 + K:o0 + K + LSV],
                        op=AO.subtract)
                    # o = min(x_ctr, u) = x - relu(d), in place over u
                    nc.vector.tensor_tensor(
                        uv, x_t[:, o0 + K:o0 + K + LSV], uv, op=AO.min)
                    nc.gpsimd.dma_start(
                        out_ap(o_d, b0 + s * BPS, BPS),
                        u3[:, s * BPS:(s + 1) * BPS, 0:FIB])
    nc.finalize()
    return nc


def _pad_inputs(x, param):
    # -> per-core padded bf16 slabs, shape [NCORES, BPC, NP, K]
    import ml_dtypes

    bf = ml_dtypes.bfloat16
    x = np.ascontiguousarray(x, dtype=np.float32).reshape(NCORES, BPC, N, K)
    param = np.ascontiguousarray(param, dtype=np.float32).reshape(NCORES, BPC, N, K)
    xp = np.empty((NCORES, BPC, NP, K), dtype=bf)
    pp = np.empty((NCORES, BPC, NP, K), dtype=bf)
    xp[:, :, 1:N + 1] = x.astype(bf)
    xp[:, :, 0] = bf(BIG)
    xp[:, :, N + 1] = bf(BIG)
    pp[:, :, 1:N + 1] = param.astype(bf)
    pp[:, :, 0] = 0.0
    pp[:, :, N + 1] = 0.0
    return xp, pp


def kernel(x: np.ndarray, param: np.ndarray) -> np.ndarray:
    global LAST_RESULTS
    from concourse.bass_utils import run_bass_kernel_spmd

    if "nc" not in _cache:
        _cache["nc"] = _build_nc()
    nc = _cache["nc"]

    xp, pp = _pad_inputs(x, param)
    in_maps = [{"x": xp[c], "p": pp[c]} for c in range(NCORES)]

    trace = bool(os.environ.get("BASS_TRACE"))
    res = run_bass_kernel_spmd(
        nc, in_maps, core_ids=list(range(NCORES)), trace=trace
    )
    LAST_RESULTS = res
    out = np.concatenate([res.results[c]["o"] for c in range(NCORES)], axis=0)
    return out.reshape(B, N, K).astype(np.float32)


# revision 14
# speedup vs baseline: 1.9946x; 1.0584x over previous
"""Convex_f forward on 8 trn2 NeuronCores (pure data parallel over batch).

Math: with y = x + param and the interior 3-point stencils
  Dy[i]    = -y[i-1] + 2 y[i] - y[i+1]          (0 at i = 0, N-1)
  mid_y[i] = 0.5 (y[i-1] + y[i+1])
the reference computes out = y - (Dy > 0) * (y - mid_y) - param.
Since y - mid_y = 0.5 * Dy on the interior, this collapses to
  out[i] = x[i] - relu(ctr - 0.5*up - 0.5*dn)   for 0 < i < N-1
  out[i] = x[i]                                  at i = 0, N-1,
and with u = x_ctr - d = 0.5*y_up + 0.5*y_dn - p_ctr it is
  out = min(x_ctr, u).

The boundary case is folded into the interior formula by padding each
batch with a halo row at both N-ends host-side: x_halo = +1e30 and
param_halo = 0, so y_halo = +1e30 and u = 0.5e30 > x at rows 0, N-1.

The problem is HBM-bound (the only real lever is bytes moved), and the
rel-err budget (2e-2) dwarfs bf16 rounding (~4e-3), so all device I/O
is bf16: host downcasts the padded inputs, device stores bf16 output,
host upcasts. 24 MiB/core of traffic vs 48 MiB in f32.

Per-core layout: partition p holds J=64 consecutive n-rows (x16 K) per
batch, so the stencil shift is a free-dim offset of K elements. The
host pre-permutes each core's slab to partition-major [P, BPC, (J+2)*K]
(halo rows duplicated across partitions), so every DMA is one 8KiB+
contiguous run per partition -- 128 descriptors per load instead of
512, ~0.7us HWDGE descriptor generation instead of ~2.5us, and >4KiB
packets at SDMA line rate. Per-batch haloed blocks of 1056 elems are
packed back-to-back in SBUF, which makes the three stencil ops
single fused DVE instructions (operands step-1 and 4B-aligned -> 2x
packed bf16 mode): positions whose stencil window straddles a block
boundary compute garbage that is simply never stored.

Engine split (strategy b16pe, default):
  PE     h = 0.5*(x + p) per 512-col chunk into PSUM (0.5*I matmul; the
         0.5 lives in the weights because DVE scalar_tensor_tensor runs
         at 1x -- only plain tensor_tensor ops get the 2x packed mode)
  ACT    downcast-copy PSUM h -> bf16 SBUF
  DVE    t = h_up + h_dn ; u = t - p_ctr ; o = min(x_ctr, u)
  SP     x AND p loads (a load issued on the ACT ring would queue
         behind the semaphore-waiting copies -- in-order sequencer --
         and serialize the load pipeline behind compute)
  GpSimd SWDGE stores (DVE runs 1-port mode, no descriptor-ring lock)
The DVE stencil + store are sub-chunked (CHUNKS per iteration) so the
first store issues before the whole iteration's stencil is done and the
pipeline tail stays short.
Strategy b16dve instead does y = x + p on DVE (no PE/ACT/PSUM).
"""

import os

import numpy as np

B, N, K = 256, 8192, 16
NCORES = 8
BPC = B // NCORES  # 32 batches per core
P = 128
J = N // P         # 64 n-rows per partition per batch
NP = N + 2         # padded rows per batch
FHB = (J + 2) * K  # 1056 haloed free elems per batch per partition
FIB = J * K        # 1024 interior free elems per batch per partition
BIG = 1.0e30

STRATEGY = os.environ.get("CONVEX_STRATEGY", "b16pe")
BPI = int(os.environ.get("CONVEX_BPI", "4"))     # batches per iteration
BUFS = int(os.environ.get("CONVEX_BUFS", "5"))
CHUNKS = int(os.environ.get("CONVEX_CHUNKS", "2"))  # stencil+store splits

_cache = {}

# Results of the last hardware run (BassKernelResults); test harnesses can
# read exec_time_ns etc. from here after calling kernel().
LAST_RESULTS = None


def _build_nc():
    import ml_dtypes
    import concourse.bacc as bacc
    import concourse.bass as bass
    import concourse.mybir as mybir
    from concourse.tile import TileContext

    bf16 = mybir.dt.bfloat16
    f32 = mybir.dt.float32
    AO = mybir.AluOpType
    L = BPI * FHB           # packed haloed elems per iteration
    BPS = BPI // CHUNKS     # batches per stencil/store sub-chunk
    LS = BPS * FHB          # packed elems per sub-chunk
    LSV = LS - 2 * K        # valid length per sub-chunk
    n_iter = BPC // BPI

    nc = bacc.Bacc()
    x_d = nc.dram_tensor("x", [P, BPC * FHB], bf16, kind="ExternalInput")
    p_d = nc.dram_tensor("p", [P, BPC * FHB], bf16, kind="ExternalInput")
    o_d = nc.dram_tensor("o", [P, BPC * FIB], bf16, kind="ExternalOutput")

    def halo_ap(handle, b0):
        # partition-major HBM layout: one contiguous BPI*FHB-elem run
        # per partition (host already duplicated the halo rows)
        return bass.AP(handle, b0 * FHB,
                       [[BPC * FHB, P], [1, BPI * FHB]])

    def out_ap(handle, b0, nb):
        return bass.AP(handle, b0 * FIB,
                       [[BPC * FIB, P], [1, nb * FIB]])

    use_pe = STRATEGY == "b16pe"
    if use_pe:
        ident_np = (0.5 * np.eye(P)).astype(ml_dtypes.bfloat16)
        ident_d = nc.inline_tensor(ident_np, name="ident")

    with TileContext(nc) as tc:
        with (
            tc.tile_pool(name="const", bufs=1) as cpool,
            tc.tile_pool(name="io", bufs=BUFS) as pool,
            tc.tile_pool(name="ps", bufs=8, space="PSUM") as pspool,
        ):
            if use_pe:
                ident_t = cpool.tile([P, P], bf16, name="ident_t")
                nc.sync.dma_start(ident_t[:], ident_d.ap())

            for it in range(n_iter):
                b0 = it * BPI
                x_t = pool.tile([P, L], bf16, name="x_t")
                p_t = pool.tile([P, L], bf16, name="p_t")
                y_t = pool.tile([P, L], bf16, name="y_t")
                u_t = pool.tile([P, L], bf16, name="u_t")

                nc.sync.dma_start(x_t[:], halo_ap(x_d, b0))
                nc.sync.dma_start(p_t[:], halo_ap(p_d, b0))

                if use_pe:
                    # h = 0.5*(x + p) on the PE (0.5*I matmul into f32
                    # PSUM), downcast to bf16 SBUF on ACT per 512-chunk
                    for c0 in range(0, L, 512):
                        c1 = min(c0 + 512, L)
                        ps = pspool.tile([P, c1 - c0], f32, name="ps")
                        nc.tensor.matmul(ps[:], ident_t[:], x_t[:, c0:c1],
                                         start=True, stop=False)
                        nc.tensor.matmul(ps[:], ident_t[:], p_t[:, c0:c1],
                                         start=False, stop=True)
                        nc.scalar.copy(y_t[:, c0:c1], ps[:])
                else:
                    # all-DVE fallback: h = 0.5*(x + p) via 2x add +
                    # 4x-capable scalar multiply (STT stencil runs 1x)
                    nc.vector.tensor_tensor(y_t[:], x_t[:], p_t[:],
                                            op=AO.add)
                    nc.vector.tensor_scalar(y_t[:], y_t[:], 0.5,
                                            op=AO.mult)

                u3 = u_t.rearrange("p (q f) -> p q f", q=BPI)
                for s in range(CHUNKS):
                    o0 = s * LS
                    # fused stencil per sub-chunk (operands bf16, step-1,
                    # 4B-aligned, plain TT -> DVE 2x packed mode);
                    # block-straddling garbage is never stored
                    uv = u_t[:, o0:o0 + LSV]
                    # t = h_up + h_dn = 0.5*(y_up + y_dn)
                    nc.vector.tensor_tensor(
                        uv, y_t[:, o0:o0 + LSV],
                        y_t[:, o0 + 2 * K:o0 + 2 * K + LSV], op=AO.add)
                    # u = t - p_ctr  ( = x_ctr - d )
                    nc.vector.tensor_tensor(
                        uv, uv, p_t[:, o0 + K:o0 + K + LSV],
                        op=AO.subtract)
                    # o = min(x_ctr, u) = x - relu(d), in place over u
                    nc.vector.tensor_tensor(
                        uv, x_t[:, o0 + K:o0 + K + LSV], uv, op=AO.min)
                    nc.gpsimd.dma_start(
                        out_ap(o_d, b0 + s * BPS, BPS),
                        u3[:, s * BPS:(s + 1) * BPS, 0:FIB])
    nc.finalize()
    return nc


def _pad_inputs(x, param):
    # -> per-core padded partition-major bf16 slabs [NCORES, P, BPC*FHB]:
    # slab[c, p, b*FHB + r*K + k] = padded[c, b, p*J + r, k], r in [0, J+2)
    import ml_dtypes
    from numpy.lib.stride_tricks import as_strided

    bf = ml_dtypes.bfloat16
    x = np.ascontiguousarray(x, dtype=np.float32).reshape(NCORES, BPC, N, K)
    param = np.ascontiguousarray(param, dtype=np.float32).reshape(NCORES, BPC, N, K)

    def prep(a, halo):
        pad = np.empty((NCORES, BPC, NP, K), dtype=bf)
        pad[:, :, 1:N + 1] = a.astype(bf)
        pad[:, :, 0] = halo
        pad[:, :, N + 1] = halo
        sc, sb, sr, sk = pad.strides
        v = as_strided(pad, shape=(NCORES, P, BPC, (J + 2) * K),
                       strides=(sc, J * sr, sb, sk))
        return np.ascontiguousarray(v).reshape(NCORES, P, BPC * FHB)

    return prep(x, bf(BIG)), prep(param, 0.0)


def kernel(x: np.ndarray, param: np.ndarray) -> np.ndarray:
    global LAST_RESULTS
    from concourse.bass_utils import run_bass_kernel_spmd

    if "nc" not in _cache:
        _cache["nc"] = _build_nc()
    nc = _cache["nc"]

    xp, pp = _pad_inputs(x, param)
    in_maps = [{"x": xp[c], "p": pp[c]} for c in range(NCORES)]

    trace = bool(os.environ.get("BASS_TRACE"))
    res = run_bass_kernel_spmd(
        nc, in_maps, core_ids=list(range(NCORES)), trace=trace
    )
    LAST_RESULTS = res
    # o[c] is [P, BPC*FIB] partition-major; un-permute to [BPC, N, K]
    out = np.stack([res.results[c]["o"] for c in range(NCORES)])
    out = out.reshape(NCORES, P, BPC, J * K).transpose(0, 2, 1, 3)
    return out.reshape(B, N, K).astype(np.float32)
